# revision 1
# baseline (speedup 1.0000x reference)
"""Masked dot-product attention (B=32, L=1024, D=64) on 8 Trainium2 NeuronCores.

Strategy: data-parallel over batches (4 per core). Per batch on-device:
  S = Q @ K^T + mask  (fp32r matmuls, mask/scale baked in as a 65th
      contraction row built host-side)
  E = exp(S/8)        (ScalarE, PSUM->SBUF, row-sums via accum_out)
  W = E * (1/rowsum)  (VectorE; written out as attn_w)
  O = W @ V           (PE-transpose E chunks to bf16 E^T, then accumulate
                       matmuls against bf16 V; normalize at the end)
Host side only reshapes/transposes/shards numpy data; all numerics that
produce the outputs run on the NeuronCores.
"""
import sys
import numpy as np
import ml_dtypes
from contextlib import ExitStack

if '/opt/trn_rl_repo' not in sys.path:
    sys.path.insert(0, '/opt/trn_rl_repo')

import concourse.bass as bass
import concourse.mybir as mybir
import concourse.tile as tile
from concourse import bacc
from concourse.bass_utils import run_bass_kernel_spmd
from concourse.masks import make_identity

FP32 = mybir.dt.float32
F32R = mybir.dt.float32r
BF16 = mybir.dt.bfloat16

B, L, D = 32, 1024, 64
NCORES = 8
BPC = B // NCORES          # batches per core
QT = L // 128              # q tiles per batch
KT = L // 128              # k tiles per batch
MASK_VALUE = -1000000.0

_cache = {}


def _emit(tc, qt_d, kt_d, v_d, w_d, o_d):
    nc = tc.nc
    Exp = mybir.ActivationFunctionType.Exp
    with ExitStack() as ctx:
        const = ctx.enter_context(tc.tile_pool(name="const", bufs=1))
        ident = const.tile([128, 128], BF16, name="ident")
        make_identity(nc, ident[:])

        qk_pool = ctx.enter_context(tc.tile_pool(name="qk", bufs=2))
        v_pool = ctx.enter_context(tc.tile_pool(name="vp", bufs=2))
        e_pool = ctx.enter_context(tc.tile_pool(name="ep", bufs=6))
        eb_pool = ctx.enter_context(tc.tile_pool(name="ebp", bufs=QT + 3))
        ebt_pool = ctx.enter_context(tc.tile_pool(name="ebtp", bufs=KT + 3))
        acc_pool = ctx.enter_context(tc.tile_pool(name="accp", bufs=4 * QT))
        osb_pool = ctx.enter_context(tc.tile_pool(name="osbp", bufs=4))
        s_ps = ctx.enter_context(tc.tile_pool(name="sps", bufs=2, space="PSUM"))
        t_ps = ctx.enter_context(tc.tile_pool(name="tps", bufs=2, space="PSUM"))
        o_ps = ctx.enter_context(tc.tile_pool(name="ops", bufs=2, space="PSUM"))

        for b in range(BPC):
            qt_sb = qk_pool.tile([65, L], F32R, name=f"qt{b}", tag="qt")
            nc.sync.dma_start(qt_sb[:], qt_d[b])
            kt_sb = qk_pool.tile([65, L], F32R, name=f"kt{b}", tag="kt")
            nc.sync.dma_start(kt_sb[:], kt_d[b])
            v_sb = v_pool.tile([128, KT * D], BF16, name=f"v{b}", tag="v")
            nc.sync.dma_start(
                v_sb[:], v_d[b].rearrange("(k p) d -> p k d", p=128))

            ebs, recs = [], []
            for q in range(QT):
                sp = s_ps.tile([128, L], FP32, name=f"s{b}_{q}", tag="s")
                lhs = qt_sb[:, q * 128:(q + 1) * 128]
                nc.tensor.matmul(sp[:, 0:512], lhs, kt_sb[:, 0:512],
                                 start=True, stop=True)
                nc.tensor.matmul(sp[:, 512:1024], lhs, kt_sb[:, 512:1024],
                                 start=True, stop=True)
                e = e_pool.tile([128, L], FP32, name=f"e{b}_{q}", tag="e")
                acc = acc_pool.tile([128, 1], FP32, name=f"a{b}_{q}", tag="a")
                nc.scalar.activation(e[:], sp[:], Exp, scale=0.125,
                                     accum_out=acc[:])
                rec = acc_pool.tile([128, 1], FP32, name=f"r{b}_{q}", tag="r")
                nc.vector.reciprocal(rec[:], acc[:])
                eb = eb_pool.tile([128, L], BF16, name=f"eb{b}_{q}", tag="eb")
                nc.gpsimd.tensor_copy(eb[:], e[:])
                # normalize in place -> W rows, then write out
                nc.vector.tensor_scalar_mul(e[:], e[:], rec[:])
                nc.sync.dma_start(w_d[b, q * 128:(q + 1) * 128, :], e[:])
                ebs.append(eb)
                recs.append(rec)

            ebts = []
            for k in range(KT):
                tp = t_ps.tile([128, L], BF16, name=f"tp{b}_{k}", tag="tp")
                for q in range(QT):
                    nc.tensor.transpose(tp[:, q * 128:(q + 1) * 128],
                                        ebs[q][:, k * 128:(k + 1) * 128],
                                        ident[:])
                ebt = ebt_pool.tile([128, L], BF16, name=f"ebt{b}_{k}", tag="ebt")
                nc.vector.tensor_copy(ebt[:], tp[:])
                ebts.append(ebt)

            for q in range(QT):
                op = o_ps.tile([128, D], FP32, name=f"op{b}_{q}", tag="op")
                for k in range(KT):
                    nc.tensor.matmul(op[:],
                                     ebts[k][:, q * 128:(q + 1) * 128],
                                     v_sb[:, k * D:(k + 1) * D],
                                     start=(k == 0), stop=(k == KT - 1))
                osb = osb_pool.tile([128, D], FP32, name=f"o{b}_{q}", tag="o")
                nc.vector.tensor_scalar_mul(osb[:], op[:], recs[q][:])
                nc.sync.dma_start(o_d[b, q * 128:(q + 1) * 128, :], osb[:])


def _build():
    if "nc" in _cache:
        return _cache["nc"]
    nc = bacc.Bacc("TRN2", debug=False, num_devices=NCORES)
    qt_d = nc.dram_tensor("qt_in", [BPC, 65, L], F32R, kind="ExternalInput").ap()
    kt_d = nc.dram_tensor("kt_in", [BPC, 65, L], F32R, kind="ExternalInput").ap()
    v_d = nc.dram_tensor("v_in", [BPC, L, D], BF16, kind="ExternalInput").ap()
    w_d = nc.dram_tensor("w_out", [BPC, L, L], FP32, kind="ExternalOutput").ap()
    o_d = nc.dram_tensor("o_out", [BPC, L, D], FP32, kind="ExternalOutput").ap()
    with tile.TileContext(nc) as tc:
        _emit(tc, qt_d, kt_d, v_d, w_d, o_d)
    nc.compile()
    _cache["nc"] = nc
    return nc


def _prep(queries, keys, values, valid_lens):
    q = np.asarray(queries, dtype=np.float32)
    k = np.asarray(keys, dtype=np.float32)
    v = np.asarray(values, dtype=np.float32)
    vl = np.asarray(valid_lens, dtype=np.int32)

    # [B, 65, L]: rows 0..63 = X^T, row 64 = ones (q side) / mask row (k side)
    qt = np.empty((B, 65, L), dtype=np.float32)
    qt[:, :64, :] = q.transpose(0, 2, 1)
    qt[:, 64, :] = 1.0
    kt = np.empty((B, 65, L), dtype=np.float32)
    kt[:, :64, :] = k.transpose(0, 2, 1)
    kt[:, 64, :] = np.where(np.arange(L)[None, :] < vl[:, None], 0.0,
                            MASK_VALUE)
    vb = v.astype(ml_dtypes.bfloat16)
    return qt, kt, vb


def kernel(queries, keys, values, valid_lens, _want_time=False):
    nc = _build()
    qt, kt, vb = _prep(queries, keys, values, valid_lens)
    in_maps = []
    for c in range(NCORES):
        s = slice(c * BPC, (c + 1) * BPC)
        in_maps.append({
            "qt_in": np.ascontiguousarray(qt[s]),
            "kt_in": np.ascontiguousarray(kt[s]),
            "v_in": np.ascontiguousarray(vb[s]).view(np.uint16),
        })
    res = run_bass_kernel_spmd(nc, in_maps, list(range(NCORES)),
                               trace=_want_time)
    attn_score = np.empty((B, L, D), dtype=np.float32)
    attn_w = np.empty((B, L, L), dtype=np.float32)
    for c in range(NCORES):
        s = slice(c * BPC, (c + 1) * BPC)
        attn_score[s] = res.results[c]["o_out"]
        attn_w[s] = res.results[c]["w_out"]
    if _want_time:
        return (attn_score, attn_w), res
    return (attn_score, attn_w)


# revision 4
# speedup vs baseline: 1.2601x; 1.2601x over previous
"""Masked dot-product attention (B=32, L=1024, D=64) on 8 Trainium2 NeuronCores.

Sharding: data-parallel over the batch dim, 4 batches per core.

Per batch, on device (everything fp32r = tf32-class matmuls, fp32 elsewhere):
  S    = Q K^T + mask     2 matmuls per 128-row q-tile; the valid_lens mask
                          and the q-side ones vector are baked host-side into
                          a 65th contraction row.
  E    = exp(S/8)         ScalarE, PSUM->SBUF (scale folded into activation)
  S^T  = K Q^T + mask^T   same operand tiles with lhsT/rhs swapped
  E^T  = exp(S^T/8)       fp32r output, feeds the O matmuls directly
  O'^T = [V|1]^T E^T      16 accumulating matmuls -> PSUM [65, 1024];
                          row 64 = softmax row-sums (ones column trick)
  O^T -> PE-transpose ->  [128, 65] tiles: col 64 = row sums in the right
                          (per-partition) layout; reciprocal + normalize
  W    = E * recip        VectorE tensor_scalar; DMA out (attn_w)
  O    = O' * recip       VectorE; DMA out (attn_score)

Host side only reshapes/transposes/shards numpy data; all output numerics
run on the NeuronCores.
"""
import sys
import numpy as np
from contextlib import ExitStack

if '/opt/trn_rl_repo' not in sys.path:
    sys.path.insert(0, '/opt/trn_rl_repo')

import concourse.bass as bass
import concourse.mybir as mybir
import concourse.tile as tile
from concourse import bacc
from concourse.bass_utils import run_bass_kernel_spmd
from concourse.masks import make_identity

FP32 = mybir.dt.float32
F32R = mybir.dt.float32r

B, L, D = 32, 1024, 64
NCORES = 8
BPC = B // NCORES          # batches per core
NT = L // 128              # 128-row tiles per batch
MASK_VALUE = -1000000.0

_cache = {}


def _emit(tc, qt_d, kt_d, v_d, w_d, o_d):
    nc = tc.nc
    Exp = mybir.ActivationFunctionType.Exp
    dma_engines = [nc.sync, nc.scalar, nc.gpsimd]

    with ExitStack() as ctx:
        const = ctx.enter_context(tc.tile_pool(name="const", bufs=1))
        ident = const.tile([128, 128], FP32, name="ident")
        make_identity(nc, ident[:])

        qk_pool = ctx.enter_context(tc.tile_pool(name="qk", bufs=2))
        v_pool = ctx.enter_context(tc.tile_pool(name="vp", bufs=2))
        e_pool = ctx.enter_context(tc.tile_pool(name="ep", bufs=NT + 3))
        et_pool = ctx.enter_context(tc.tile_pool(name="etp", bufs=NT + 3))
        w_pool = ctx.enter_context(tc.tile_pool(name="wp", bufs=4))
        ot_pool = ctx.enter_context(tc.tile_pool(name="otp", bufs=2))
        acc_pool = ctx.enter_context(tc.tile_pool(name="accp", bufs=4 * NT))
        osb_pool = ctx.enter_context(tc.tile_pool(name="osbp", bufs=4))
        # PSUM budget = 8 banks: sA 2x[128,1024] = 4, sT/bk shared 2x1 = 2,
        # oT 2x[65,512] = 2.
        sA_ps = ctx.enter_context(tc.tile_pool(name="sAps", bufs=2, space="PSUM"))
        sT_ps = ctx.enter_context(tc.tile_pool(name="sTps", bufs=2, space="PSUM"))
        oT_ps = ctx.enter_context(tc.tile_pool(name="oTps", bufs=2, space="PSUM"))

        state = {}

        def phase_load(b):
            qt_sb = qk_pool.tile([65, L], F32R, name=f"qt{b}", tag="qt")
            nc.gpsimd.dma_start(qt_sb[:], qt_d[b])
            kt_sb = qk_pool.tile([65, L], F32R, name=f"kt{b}", tag="kt")
            nc.gpsimd.dma_start(kt_sb[:], kt_d[b])
            v_sb = v_pool.tile([128, NT * 65], F32R, name=f"v{b}", tag="v")
            nc.gpsimd.dma_start(
                v_sb[:], v_d[b].rearrange("(k p) d -> p k d", p=128))
            state[b] = dict(qt=qt_sb, kt=kt_sb, v=v_sb, e=[], et=[])

        def phase_s(b):
            st = state[b]
            for q in range(NT):
                sp = sA_ps.tile([128, L], FP32, name=f"s{b}_{q}", tag="s")
                lhs = st["qt"][:, q * 128:(q + 1) * 128]
                nc.tensor.matmul(sp[:, 0:512], lhs, st["kt"][:, 0:512],
                                 start=True, stop=True)
                nc.tensor.matmul(sp[:, 512:1024], lhs, st["kt"][:, 512:1024],
                                 start=True, stop=True)
                e = e_pool.tile([128, L], FP32, name=f"e{b}_{q}", tag="e")
                nc.scalar.activation(e[:], sp[:], Exp, scale=0.125)
                st["e"].append(e)

        def phase_st(b):
            st = state[b]
            for k in range(NT):
                et = et_pool.tile([128, L], F32R, name=f"et{b}_{k}", tag="et")
                lhs = st["kt"][:, k * 128:(k + 1) * 128]
                for h in range(2):
                    sp = sT_ps.tile([128, 512], FP32,
                                    name=f"t{b}_{k}_{h}", tag="t")
                    nc.tensor.matmul(sp[:], lhs,
                                     st["qt"][:, h * 512:(h + 1) * 512],
                                     start=True, stop=True)
                    nc.scalar.activation(et[:, h * 512:(h + 1) * 512], sp[:],
                                         Exp, scale=0.125)
                st["et"].append(et)

        def phase_o(b):
            st = state[b]
            ot_sb = ot_pool.tile([65, L], FP32, name=f"otsb{b}", tag="otsb")
            for h in range(2):
                op = oT_ps.tile([65, 512], FP32, name=f"ot{b}_{h}", tag="ot")
                for k in range(NT):
                    lhs = st["v"][:, k * 65:(k + 1) * 65]
                    nc.tensor.matmul(op[:], lhs,
                                     st["et"][k][:, h * 512:(h + 1) * 512],
                                     start=(k == 0), stop=(k == NT - 1))
                nc.vector.tensor_copy(ot_sb[:, h * 512:(h + 1) * 512], op[:])
            st["ot"] = ot_sb

        def phase_out(b):
            st = state[b]
            for q in range(NT):
                bk = sT_ps.tile([128, 65], FP32, name=f"bk{b}_{q}", tag="t")
                nc.tensor.transpose(bk[:], st["ot"][:, q * 128:(q + 1) * 128],
                                    ident[0:65, 0:65])
                rec = acc_pool.tile([128, 1], FP32, name=f"r{b}_{q}", tag="r")
                nc.vector.reciprocal(rec[:], bk[:, 64:65])
                osb = osb_pool.tile([128, D], FP32, name=f"o{b}_{q}", tag="o")
                nc.vector.tensor_scalar_mul(osb[:], bk[:, 0:D], rec[:])
                dma_engines[q % 3].dma_start(
                    o_d[b, q * 128:(q + 1) * 128, :], osb[:])
                w = w_pool.tile([128, L], FP32, name=f"w{b}_{q}", tag="w")
                nc.vector.tensor_scalar_mul(w[:], st["e"][q][:], rec[:])
                dma_engines[(q + 1) % 3].dma_start(
                    w_d[b, q * 128:(q + 1) * 128, :], w[:])

        # skewed pipeline: keep PE busy while previous batch's output drains
        phase_load(0)
        phase_s(0)
        for b in range(BPC):
            if b + 1 < BPC:
                phase_load(b + 1)
            phase_st(b)
            if b + 1 < BPC:
                phase_s(b + 1)
            phase_o(b)
            phase_out(b)


def _build():
    if "nc" in _cache:
        return _cache["nc"]
    nc = bacc.Bacc("TRN2", debug=False, num_devices=NCORES)
    qt_d = nc.dram_tensor("qt_in", [BPC, 65, L], F32R, kind="ExternalInput").ap()
    kt_d = nc.dram_tensor("kt_in", [BPC, 65, L], F32R, kind="ExternalInput").ap()
    v_d = nc.dram_tensor("v_in", [BPC, L, 65], F32R, kind="ExternalInput").ap()
    w_d = nc.dram_tensor("w_out", [BPC, L, L], FP32, kind="ExternalOutput").ap()
    o_d = nc.dram_tensor("o_out", [BPC, L, D], FP32, kind="ExternalOutput").ap()
    with tile.TileContext(nc) as tc:
        _emit(tc, qt_d, kt_d, v_d, w_d, o_d)
    nc.compile()
    _cache["nc"] = nc
    return nc


def _prep(queries, keys, values, valid_lens):
    q = np.asarray(queries, dtype=np.float32)
    k = np.asarray(keys, dtype=np.float32)
    v = np.asarray(values, dtype=np.float32)
    vl = np.asarray(valid_lens, dtype=np.int32)

    # [B, 65, L]: rows 0..63 = X^T; row 64 = ones (q side) / mask row (k side)
    qt = np.empty((B, 65, L), dtype=np.float32)
    qt[:, :64, :] = q.transpose(0, 2, 1)
    qt[:, 64, :] = 1.0
    kt = np.empty((B, 65, L), dtype=np.float32)
    kt[:, :64, :] = k.transpose(0, 2, 1)
    kt[:, 64, :] = np.where(np.arange(L)[None, :] < vl[:, None], 0.0,
                            MASK_VALUE)
    # [B, L, 65]: [V | 1] — the ones column yields softmax row-sums
    vp = np.empty((B, L, 65), dtype=np.float32)
    vp[:, :, :64] = v
    vp[:, :, 64] = 1.0
    return qt, kt, vp


def kernel(queries, keys, values, valid_lens, _want_time=False):
    nc = _build()
    qt, kt, vp = _prep(queries, keys, values, valid_lens)
    in_maps = []
    for c in range(NCORES):
        s = slice(c * BPC, (c + 1) * BPC)
        in_maps.append({
            "qt_in": np.ascontiguousarray(qt[s]),
            "kt_in": np.ascontiguousarray(kt[s]),
            "v_in": np.ascontiguousarray(vp[s]),
        })
    res = run_bass_kernel_spmd(nc, in_maps, list(range(NCORES)),
                               trace=_want_time)
    attn_score = np.empty((B, L, D), dtype=np.float32)
    attn_w = np.empty((B, L, L), dtype=np.float32)
    for c in range(NCORES):
        s = slice(c * BPC, (c + 1) * BPC)
        attn_score[s] = res.results[c]["o_out"]
        attn_w[s] = res.results[c]["w_out"]
    if _want_time:
        return (attn_score, attn_w), res
    return (attn_score, attn_w)


# revision 6
# speedup vs baseline: 1.5927x; 1.2639x over previous
"""Masked dot-product attention (B=32, L=1024, D=64) on 8 Trainium2 NeuronCores.

Sharding: data-parallel over the batch dim, 4 batches per core.

Per batch, on device (fp32r = tf32-class matmuls, fp32 elsewhere):
  S    = Q K^T + mask     2 matmuls per 128-row q-tile; the valid_lens mask
                          and the q-side ones vector are baked host-side into
                          a 65th contraction row.
  E    = exp(S/8)         ScalarE, PSUM->SBUF (scale folded into activation)
  S^T  = K Q^T + mask^T   same operand tiles with lhsT/rhs swapped
  E^T  = exp(S^T/8)       fp32r output, feeds the O matmuls directly
  O'^T = [V|1]^T E^T      accumulating matmuls -> PSUM [65, 512] halves;
                          row 64 = softmax row-sums (ones column trick)
  O^T -> PE-transpose ->  [128, 65] tiles: col 64 = row sums in the right
                          (per-partition) layout; reciprocal + normalize
  W    = E * recip        VectorE tensor_scalar; one 4 MB DMA per batch
  O    = O' * recip       VectorE; one DMA per batch

The q rows are processed in a permuted order (q = 8*p + r for tile r,
partition p) so that each partition's slice of the per-batch attn_w output
is 32 KB contiguous in DRAM (large DMA descriptors). attn_w is written in
final layout on device via a rearranged AP; attn_score is written tile-major
and un-permuted on the host (pure layout gather).

Host side only reshapes/transposes/shards numpy data; all output numerics
run on the NeuronCores.
"""
import sys
import numpy as np
from contextlib import ExitStack

if '/opt/trn_rl_repo' not in sys.path:
    sys.path.insert(0, '/opt/trn_rl_repo')

import concourse.bass as bass
import concourse.mybir as mybir
import concourse.tile as tile
from concourse import bacc
from concourse.bass_utils import run_bass_kernel_spmd
from concourse.masks import make_identity

FP32 = mybir.dt.float32
F32R = mybir.dt.float32r

B, L, D = 32, 1024, 64
NCORES = 8
BPC = B // NCORES          # batches per core
NT = L // 128              # 128-row tiles per batch
MASK_VALUE = -1000000.0

_cache = {}


def _emit(tc, qt_d, kt_d, v_d, w_d, o_d):
    nc = tc.nc
    Exp = mybir.ActivationFunctionType.Exp

    with ExitStack() as ctx:
        const = ctx.enter_context(tc.tile_pool(name="const", bufs=1))
        ident = const.tile([128, 128], FP32, name="ident")
        make_identity(nc, ident[:])

        qk_pool = ctx.enter_context(tc.tile_pool(name="qk", bufs=2))
        v_pool = ctx.enter_context(tc.tile_pool(name="vp", bufs=2))
        e_pool = ctx.enter_context(tc.tile_pool(name="ep", bufs=NT + 2))
        et_pool = ctx.enter_context(tc.tile_pool(name="etp", bufs=NT + 2))
        w_pool = ctx.enter_context(tc.tile_pool(name="wp", bufs=2))
        ot_pool = ctx.enter_context(tc.tile_pool(name="otp", bufs=2))
        acc_pool = ctx.enter_context(tc.tile_pool(name="accp", bufs=4 * NT))
        osb_pool = ctx.enter_context(tc.tile_pool(name="osbp", bufs=2))
        # PSUM (8 banks): sA shared S/S^T [128,1024] x3 = 6, oT/bk shared x2 = 2
        sA_ps = ctx.enter_context(tc.tile_pool(name="sAps", bufs=3, space="PSUM"))
        ob_ps = ctx.enter_context(tc.tile_pool(name="obps", bufs=2, space="PSUM"))

        state = {}

        def phase_load(b):
            qt_sb = qk_pool.tile([65, L], F32R, name=f"qt{b}", tag="qt")
            nc.sync.dma_start(qt_sb[:], qt_d[b])
            kt_sb = qk_pool.tile([65, L], F32R, name=f"kt{b}", tag="kt")
            nc.sync.dma_start(kt_sb[:], kt_d[b])
            v_sb = v_pool.tile([128, NT * 65], F32R, name=f"v{b}", tag="v")
            nc.sync.dma_start(
                v_sb[:], v_d[b].rearrange("(k p) d -> p k d", p=128))
            state[b] = dict(qt=qt_sb, kt=kt_sb, v=v_sb, e=[], et=[])

        def phase_s(b):
            st = state[b]
            for q in range(NT):
                sp = sA_ps.tile([128, L], FP32, name=f"s{b}_{q}", tag="s")
                lhs = st["qt"][:, q * 128:(q + 1) * 128]
                nc.tensor.matmul(sp[:, 0:512], lhs, st["kt"][:, 0:512],
                                 start=True, stop=True)
                nc.tensor.matmul(sp[:, 512:1024], lhs, st["kt"][:, 512:1024],
                                 start=True, stop=True)
                e = e_pool.tile([128, L], FP32, name=f"e{b}_{q}", tag="e")
                nc.scalar.activation(e[:], sp[:], Exp, scale=0.125)
                st["e"].append(e)

        def phase_st(b):
            st = state[b]
            for k in range(NT):
                sp = sA_ps.tile([128, L], FP32, name=f"t{b}_{k}", tag="s")
                lhs = st["kt"][:, k * 128:(k + 1) * 128]
                nc.tensor.matmul(sp[:, 0:512], lhs, st["qt"][:, 0:512],
                                 start=True, stop=True)
                nc.tensor.matmul(sp[:, 512:1024], lhs, st["qt"][:, 512:1024],
                                 start=True, stop=True)
                et = et_pool.tile([128, L], F32R, name=f"et{b}_{k}", tag="et")
                nc.scalar.activation(et[:], sp[:], Exp, scale=0.125)
                st["et"].append(et)

        def phase_o(b):
            st = state[b]
            ot_sb = ot_pool.tile([65, L], FP32, name=f"otsb{b}", tag="otsb")
            for h in range(2):
                op = ob_ps.tile([65, 512], FP32, name=f"ot{b}_{h}", tag="ob")
                for k in range(NT):
                    lhs = st["v"][:, k * 65:(k + 1) * 65]
                    nc.tensor.matmul(op[:], lhs,
                                     st["et"][k][:, h * 512:(h + 1) * 512],
                                     start=(k == 0), stop=(k == NT - 1))
                nc.vector.tensor_copy(ot_sb[:, h * 512:(h + 1) * 512], op[:])
            st["ot"] = ot_sb

        def phase_out(b):
            st = state[b]
            w = w_pool.tile([128, NT * L], FP32, name=f"w{b}", tag="w")
            osb = osb_pool.tile([128, NT * D], FP32, name=f"osb{b}", tag="o")
            for q in range(NT):
                bk = ob_ps.tile([128, 65], FP32, name=f"bk{b}_{q}", tag="ob")
                nc.tensor.transpose(bk[:], st["ot"][:, q * 128:(q + 1) * 128],
                                    ident[0:65, 0:65])
                rec = acc_pool.tile([128, 1], FP32, name=f"r{b}_{q}", tag="r")
                nc.vector.reciprocal(rec[:], bk[:, 64:65])
                nc.vector.tensor_scalar_mul(osb[:, q * D:(q + 1) * D],
                                            bk[:, 0:D], rec[:])
                nc.vector.tensor_scalar_mul(w[:, q * L:(q + 1) * L],
                                            st["e"][q][:], rec[:])
            nc.sync.dma_start(w_d[b].rearrange("(p r) k -> p (r k)", r=NT),
                              w[:])
            nc.sync.dma_start(o_d[b].rearrange("r p d -> p r d"),
                              osb[:].rearrange("p (r d) -> p r d", r=NT))

        # skewed pipeline: keep PE/ACT busy while previous batch drains
        phase_load(0)
        phase_s(0)
        for b in range(BPC):
            if b + 1 < BPC:
                phase_load(b + 1)
            phase_st(b)
            if b + 1 < BPC:
                phase_s(b + 1)
            phase_o(b)
            phase_out(b)


def _build():
    if "nc" in _cache:
        return _cache["nc"]
    nc = bacc.Bacc("TRN2", debug=False, num_devices=NCORES)
    qt_d = nc.dram_tensor("qt_in", [BPC, 65, L], F32R, kind="ExternalInput").ap()
    kt_d = nc.dram_tensor("kt_in", [BPC, 65, L], F32R, kind="ExternalInput").ap()
    v_d = nc.dram_tensor("v_in", [BPC, L, 65], F32R, kind="ExternalInput").ap()
    w_d = nc.dram_tensor("w_out", [BPC, L, L], FP32, kind="ExternalOutput").ap()
    o_d = nc.dram_tensor("o_out", [BPC, NT, 128, D], FP32,
                         kind="ExternalOutput").ap()
    with tile.TileContext(nc) as tc:
        _emit(tc, qt_d, kt_d, v_d, w_d, o_d)
    nc.compile()
    _cache["nc"] = nc
    return nc


def _prep(queries, keys, values, valid_lens):
    q = np.asarray(queries, dtype=np.float32)
    k = np.asarray(keys, dtype=np.float32)
    v = np.asarray(values, dtype=np.float32)
    vl = np.asarray(valid_lens, dtype=np.int32)

    # [B, 65, L]: rows 0..63 = X^T; row 64 = ones (q side) / mask row (k side)
    # q columns permuted so column c = r*128 + p maps to q = 8*p + r.
    qt = np.empty((B, 65, L), dtype=np.float32)
    qt[:, :64, :] = (q.transpose(0, 2, 1)
                     .reshape(B, 64, 128, NT).transpose(0, 1, 3, 2)
                     .reshape(B, 64, L))
    qt[:, 64, :] = 1.0
    kt = np.empty((B, 65, L), dtype=np.float32)
    kt[:, :64, :] = k.transpose(0, 2, 1)
    kt[:, 64, :] = np.where(np.arange(L)[None, :] < vl[:, None], 0.0,
                            MASK_VALUE)
    # [B, L, 65]: [V | 1] — the ones column yields softmax row-sums
    vp = np.empty((B, L, 65), dtype=np.float32)
    vp[:, :, :64] = v
    vp[:, :, 64] = 1.0
    return qt, kt, vp


def kernel(queries, keys, values, valid_lens, _want_time=False):
    nc = _build()
    qt, kt, vp = _prep(queries, keys, values, valid_lens)
    in_maps = []
    for c in range(NCORES):
        s = slice(c * BPC, (c + 1) * BPC)
        in_maps.append({
            "qt_in": np.ascontiguousarray(qt[s]),
            "kt_in": np.ascontiguousarray(kt[s]),
            "v_in": np.ascontiguousarray(vp[s]),
        })
    res = run_bass_kernel_spmd(nc, in_maps, list(range(NCORES)),
                               trace=_want_time)
    attn_score = np.empty((B, L, D), dtype=np.float32)
    attn_w = np.empty((B, L, L), dtype=np.float32)
    for c in range(NCORES):
        s = slice(c * BPC, (c + 1) * BPC)
        attn_w[s] = res.results[c]["w_out"]
        # o_out is [BPC, r, p, D] with q = 8*p + r
        o = res.results[c]["o_out"]
        attn_score[s] = o.transpose(0, 2, 1, 3).reshape(BPC, L, D)
    if _want_time:
        return (attn_score, attn_w), res
    return (attn_score, attn_w)


# revision 8
# speedup vs baseline: 1.6577x; 1.0408x over previous
"""Masked dot-product attention (B=32, L=1024, D=64) on 8 Trainium2 NeuronCores.

Sharding: data-parallel over the batch dim, 4 batches per core.

Per batch, on device (fp32r = tf32-class matmuls, fp32 elsewhere):
  S    = Q K^T + mask     2 matmuls per 128-row q-tile; the valid_lens mask
                          and the q-side ones vector are baked host-side into
                          a 65th contraction row.
  E    = exp(S/8)         ScalarE, PSUM->SBUF (scale folded into activation)
  S^T  = K Q^T + mask^T   same operand tiles with lhsT/rhs swapped
  E^T  = exp(S^T/8)       fp32r output, feeds the O matmuls directly
  O'^T = [V|1]^T E^T      accumulating matmuls -> PSUM [65, 512] halves;
                          row 64 = softmax row-sums (ones column trick)
  O^T -> PE-transpose ->  [128, 65] tiles: col 64 = row sums in the right
                          (per-partition) layout; reciprocal + normalize
  W    = E * recip        VectorE tensor_scalar; one 4 MB DMA per batch
  O    = O' * recip       VectorE; one DMA per batch

The q rows are processed in a permuted order (q = 8*p + r for tile r,
partition p) so that each partition's slice of the per-batch attn_w output
is 32 KB contiguous in DRAM (large DMA descriptors). attn_w is written in
final layout on device via a rearranged AP; attn_score is written tile-major
and un-permuted on the host (pure layout gather).

Host side only reshapes/transposes/shards numpy data; all output numerics
run on the NeuronCores.
"""
import sys
import numpy as np
from contextlib import ExitStack

if '/opt/trn_rl_repo' not in sys.path:
    sys.path.insert(0, '/opt/trn_rl_repo')

import concourse.bass as bass
import concourse.mybir as mybir
import concourse.tile as tile
from concourse import bacc
from concourse.bass_utils import run_bass_kernel_spmd
from concourse.masks import make_identity

FP32 = mybir.dt.float32
F32R = mybir.dt.float32r

B, L, D = 32, 1024, 64
NCORES = 8
BPC = B // NCORES          # batches per core
NT = L // 128              # 128-row tiles per batch
MASK_VALUE = -1000000.0

_cache = {}


def _emit(tc, qt_d, kt_d, v_d, w_d, o_d):
    nc = tc.nc
    Exp = mybir.ActivationFunctionType.Exp

    with ExitStack() as ctx:
        const = ctx.enter_context(tc.tile_pool(name="const", bufs=1))
        ident = const.tile([128, 128], FP32, name="ident")
        make_identity(nc, ident[:])

        qk_pool = ctx.enter_context(tc.tile_pool(name="qk", bufs=3))
        v_pool = ctx.enter_context(tc.tile_pool(name="vp", bufs=3))
        e_pool = ctx.enter_context(tc.tile_pool(name="ep", bufs=2 * NT + 2))
        et_pool = ctx.enter_context(tc.tile_pool(name="etp", bufs=NT + 4))
        w_pool = ctx.enter_context(tc.tile_pool(name="wp", bufs=6))
        ot_pool = ctx.enter_context(tc.tile_pool(name="otp", bufs=2))
        acc_pool = ctx.enter_context(tc.tile_pool(name="accp", bufs=4 * NT))
        osb_pool = ctx.enter_context(tc.tile_pool(name="osbp", bufs=2))
        # PSUM (8 banks): sA shared S/S^T [128,1024] x2 = 4, oT/bk shared x4 = 4
        sA_ps = ctx.enter_context(tc.tile_pool(name="sAps", bufs=2, space="PSUM"))
        ob_ps = ctx.enter_context(tc.tile_pool(name="obps", bufs=4, space="PSUM"))

        state = {}

        def load(b):
            if b >= BPC:
                return
            qt_sb = qk_pool.tile([65, L], F32R, name=f"qt{b}", tag="qt")
            nc.gpsimd.dma_start(qt_sb[:], qt_d[b])
            kt_sb = qk_pool.tile([65, L], F32R, name=f"kt{b}", tag="kt")
            nc.gpsimd.dma_start(kt_sb[:], kt_d[b])
            v_sb = v_pool.tile([128, NT * 65], F32R, name=f"v{b}", tag="v")
            nc.gpsimd.dma_start(
                v_sb[:], v_d[b].rearrange("(k p) d -> p k d", p=128))
            state[b] = dict(qt=qt_sb, kt=kt_sb, v=v_sb, e=[None] * NT,
                            et=[None] * NT)

        def s_tile(b, q):
            if b >= BPC:
                return
            st = state[b]
            sp = sA_ps.tile([128, L], FP32, name=f"s{b}_{q}", tag="s")
            lhs = st["qt"][:, q * 128:(q + 1) * 128]
            nc.tensor.matmul(sp[:, 0:512], lhs, st["kt"][:, 0:512],
                             start=True, stop=True)
            nc.tensor.matmul(sp[:, 512:1024], lhs, st["kt"][:, 512:1024],
                             start=True, stop=True)
            e = e_pool.tile([128, L], FP32, name=f"e{b}_{q}", tag="e")
            nc.scalar.activation(e[:], sp[:], Exp, scale=0.125)
            st["e"][q] = e

        def st_tile(b, k):
            if b >= BPC:
                return
            st = state[b]
            sp = sA_ps.tile([128, L], FP32, name=f"t{b}_{k}", tag="s")
            lhs = st["kt"][:, k * 128:(k + 1) * 128]
            nc.tensor.matmul(sp[:, 0:512], lhs, st["qt"][:, 0:512],
                             start=True, stop=True)
            nc.tensor.matmul(sp[:, 512:1024], lhs, st["qt"][:, 512:1024],
                             start=True, stop=True)
            et = et_pool.tile([128, L], F32R, name=f"et{b}_{k}", tag="et")
            nc.scalar.activation(et[:], sp[:], Exp, scale=0.125)
            st["et"][k] = et

        def o_mm(b, h, k):
            st = state[b]
            if k == 0:
                st.setdefault("op", {})[h] = ob_ps.tile(
                    [65, 512], FP32, name=f"ot{b}_{h}", tag="ob")
            op = st["op"][h]
            lhs = st["v"][:, k * 65:(k + 1) * 65]
            nc.tensor.matmul(op[:], lhs,
                             st["et"][k][:, h * 512:(h + 1) * 512],
                             start=(k == 0), stop=(k == NT - 1))
            if k == NT - 1:
                if "ot" not in st:
                    st["ot"] = ot_pool.tile([65, L], FP32, name=f"otsb{b}",
                                            tag="otsb")
                nc.vector.tensor_copy(
                    st["ot"][:, h * 512:(h + 1) * 512], op[:])

        def bk_tile(b, q):
            st = state[b]
            if "w" not in st:
                st["w"] = []
                st["osb"] = osb_pool.tile([128, NT * D], FP32,
                                          name=f"osb{b}", tag="o")
            bk = ob_ps.tile([128, 65], FP32, name=f"bk{b}_{q}", tag="ob")
            nc.tensor.transpose(bk[:], st["ot"][:, q * 128:(q + 1) * 128],
                                ident[0:65, 0:65])
            rec = acc_pool.tile([128, 1], FP32, name=f"r{b}_{q}", tag="r")
            nc.vector.reciprocal(rec[:], bk[:, 64:65])
            nc.vector.tensor_scalar_mul(st["osb"][:, q * D:(q + 1) * D],
                                        bk[:, 0:D], rec[:])
            w = w_pool.tile([128, L], FP32, name=f"w{b}_{q}", tag="w")
            nc.vector.tensor_scalar_mul(w[:], st["e"][q][:], rec[:])
            # tile q holds DRAM rows {NT*p + q}: stride-NT row scatter, 4KB runs
            nc.sync.dma_start(
                w_d[b].rearrange("(p r) k -> p r k", r=NT)[:, q, :], w[:])
            st["w"].append(w)

        def o_dma(b):
            st = state[b]
            nc.sync.dma_start(
                o_d[b].rearrange("r p d -> p r d"),
                st["osb"][:].rearrange("p (r d) -> p r d", r=NT))

        # --- schedule: fine-grained interleave to keep PE dense ---
        load(0)
        load(1)
        for j in range(NT):
            s_tile(0, j)
        for j in range(NT):
            st_tile(0, j)
        for b in range(BPC):
            load(b + 2)
            # O^T half 0, with next batch's S tiles woven in
            for k in range(NT):
                o_mm(b, 0, k)
                if k % 2 == 1:
                    s_tile(b + 1, k // 2)
            # O^T half 1, woven with first BK tiles and remaining S tiles
            for k in range(NT):
                o_mm(b, 1, k)
                if k % 2 == 0:
                    bk_tile(b, k // 2)
                else:
                    s_tile(b + 1, 4 + k // 2)
            # tail: remaining BK tiles woven with next batch's S^T tiles
            for j in range(NT):
                if j < 4:
                    bk_tile(b, 4 + j)
                st_tile(b + 1, j)
            o_dma(b)


def _build():
    if "nc" in _cache:
        return _cache["nc"]
    nc = bacc.Bacc("TRN2", debug=False, num_devices=NCORES)
    qt_d = nc.dram_tensor("qt_in", [BPC, 65, L], F32R, kind="ExternalInput").ap()
    kt_d = nc.dram_tensor("kt_in", [BPC, 65, L], F32R, kind="ExternalInput").ap()
    v_d = nc.dram_tensor("v_in", [BPC, L, 65], F32R, kind="ExternalInput").ap()
    w_d = nc.dram_tensor("w_out", [BPC, L, L], FP32, kind="ExternalOutput").ap()
    o_d = nc.dram_tensor("o_out", [BPC, NT, 128, D], FP32,
                         kind="ExternalOutput").ap()
    with tile.TileContext(nc) as tc:
        _emit(tc, qt_d, kt_d, v_d, w_d, o_d)
    nc.compile()
    _cache["nc"] = nc
    return nc


def _prep(queries, keys, values, valid_lens):
    q = np.asarray(queries, dtype=np.float32)
    k = np.asarray(keys, dtype=np.float32)
    v = np.asarray(values, dtype=np.float32)
    vl = np.asarray(valid_lens, dtype=np.int32)

    # [B, 65, L]: rows 0..63 = X^T; row 64 = ones (q side) / mask row (k side)
    # q columns permuted so column c = r*128 + p maps to q = 8*p + r.
    qt = np.empty((B, 65, L), dtype=np.float32)
    qt[:, :64, :] = (q.transpose(0, 2, 1)
                     .reshape(B, 64, 128, NT).transpose(0, 1, 3, 2)
                     .reshape(B, 64, L))
    qt[:, 64, :] = 1.0
    kt = np.empty((B, 65, L), dtype=np.float32)
    kt[:, :64, :] = k.transpose(0, 2, 1)
    kt[:, 64, :] = np.where(np.arange(L)[None, :] < vl[:, None], 0.0,
                            MASK_VALUE)
    # [B, L, 65]: [V | 1] — the ones column yields softmax row-sums
    vp = np.empty((B, L, 65), dtype=np.float32)
    vp[:, :, :64] = v
    vp[:, :, 64] = 1.0
    return qt, kt, vp


def kernel(queries, keys, values, valid_lens, _want_time=False):
    nc = _build()
    qt, kt, vp = _prep(queries, keys, values, valid_lens)
    in_maps = []
    for c in range(NCORES):
        s = slice(c * BPC, (c + 1) * BPC)
        in_maps.append({
            "qt_in": np.ascontiguousarray(qt[s]),
            "kt_in": np.ascontiguousarray(kt[s]),
            "v_in": np.ascontiguousarray(vp[s]),
        })
    res = run_bass_kernel_spmd(nc, in_maps, list(range(NCORES)),
                               trace=_want_time)
    attn_score = np.empty((B, L, D), dtype=np.float32)
    attn_w = np.empty((B, L, L), dtype=np.float32)
    for c in range(NCORES):
        s = slice(c * BPC, (c + 1) * BPC)
        attn_w[s] = res.results[c]["w_out"]
        # o_out is [BPC, r, p, D] with q = 8*p + r
        o = res.results[c]["o_out"]
        attn_score[s] = o.transpose(0, 2, 1, 3).reshape(BPC, L, D)
    if _want_time:
        return (attn_score, attn_w), res
    return (attn_score, attn_w)
